# revision 19
# baseline (speedup 1.0000x reference)
"""Causal self-attention on 8 TRN2 NeuronCores (pipelined bf16 rewrite).

Problem: B=4, T=2048, C=1024, NH=16, HD=64.
  qkv = x @ w_qkv ; per-head causal softmax attention ; y @ w_proj

Structure (vs the 417us half-split baseline):
  - scores for both heads of a kv-chunk land in ONE [128, 2, 512] PSUM
    tile (2 banks); a single ACT exp instruction covers both heads, which
    halves the scalar-engine instruction count (the attention pacer).
  - emission is software-pipelined: the PE queue gets score(i+1) before
    y(i), and qkv matmuls are drained a few at a time between attention
    chunks, so the PE stream stays dense; attention+qkv complete in
    ~110us of issue time.
  - attention tiles run j-major; TWO AllToAll collectives (q-tiles 0-1,
    then 2-3) move unnormalized y + s rows; each collective costs
    ~xfer + a fixed ~35us ncfw re-arm and the first one also absorbs
    the cross-core launch skew, so fewer collectives beat many.
  - all proj work runs at the tail, draining in collective-completion
    order so it fills the collective waits; output DMAs ride the scalar
    queue (idle once the exp stream ends) so they never delay staging.
  - softmax denominator: ones-column in v rides the y-matmul; 1/s via a
    [16, 256] partition-packed nc.vector.reciprocal per quarter.
"""

import numpy as np
import ml_dtypes
from collections import deque

import concourse.bass as bass
import concourse.mybir as mybir
import concourse.tile as tile
from concourse import bacc
from concourse.bass_utils import run_bass_kernel_spmd
from concourse.masks import make_identity

B, T, C = 4, 2048, 1024
NH, HD = 16, 64
NCORES = 8
HPC = NH // NCORES          # heads per core = 2
D2 = HPC * HD               # 128 head-dims per core
ROWS = B * T                # 8192 flattened rows
RSL = ROWS // NCORES        # 1024 rows per core slice
P = 128
QTL = 512                   # q tile
NJ = T // QTL               # 4 q-tiles per batch
KVC = T // P                # 16 kv chunks of 128 per batch
NRC = 4                     # row chunks (of 512) per batch for qkv
CK = C // P                 # 8 contraction chunks
QC = 256                    # cols owned per core per q-tile
SCALE = 1.0 / np.sqrt(HD)

F32 = mybir.dt.float32
BF16 = mybir.dt.bfloat16
AF = mybir.ActivationFunctionType
ALU = mybir.AluOpType

_CACHED_NC = None
LAST_RESULTS = None  # BassKernelResults of the most recent launch (for profiling)

try:  # reuse compiled executables across calls/processes when supported
    import jax

    jax.config.update("jax_compilation_cache_dir", "/tmp/jax_cache")
    jax.config.update("jax_persistent_cache_min_compile_time_secs", 1.0)
except Exception:
    pass


def _build_nc():
    nc = bacc.Bacc(None, target_bir_lowering=False, num_devices=NCORES)

    xT_in = nc.dram_tensor("xT", [C, ROWS], BF16, kind="ExternalInput")
    wq_in = nc.dram_tensor("wq", [P, CK, D2], BF16, kind="ExternalInput")
    wk_in = nc.dram_tensor("wk", [P, CK, D2], BF16, kind="ExternalInput")
    wv_in = nc.dram_tensor("wv", [P, CK, D2], BF16, kind="ExternalInput")
    wp_in = nc.dram_tensor("wp", [P, CK, C], BF16, kind="ExternalInput")
    tri_in = nc.dram_tensor("tri", [P, HPC, P], BF16, kind="ExternalInput")
    e2_in = nc.dram_tensor("e2", [NH, CK, P], BF16, kind="ExternalInput")
    outT = nc.dram_tensor("outT", [C, RSL], F32, kind="ExternalOutput")

    rg = [list(range(NCORES))]

    # two collectives; each carries two q-tile groups (j values).
    # tile (b, j) splits its 512 q-cols over slots 2b+{0,1}; core r owns
    # (b=r//2, h2=r%2) of every j.
    GROUPS = [
        ("A", (0, 1), [(b, j) for j in (0, 1) for b in range(B)]),
        ("B", (2, 3), [(b, j) for j in (2, 3) for b in range(B)]),
    ]

    with tile.TileContext(nc) as tc:
        with (
            tc.tile_pool(name="persist", bufs=1) as pp,
            tc.tile_pool(name="dram", bufs=1, space="DRAM") as dram,
            tc.tile_pool(name="work", bufs=1) as pw,
            tc.tile_pool(name="psum", bufs=1, space="PSUM") as ps,
        ):
            # ---- DRAM collective buffers ----
            # [slot, features(y 0:128 | s 128:130), j-within-group, cols]
            a_in = {}
            a_out = {}
            for gname, _, _ in GROUPS:
                a_in[gname] = dram.tile([NCORES, D2 + HPC, 2, QC], BF16,
                                        name=f"ain_{gname}")
                a_out[gname] = dram.tile([NCORES, D2 + HPC, 2, QC], BF16,
                                         name=f"aout_{gname}")

            ident = pp.tile([P, P], BF16)
            make_identity(nc, ident[:])

            tri_sb = pp.tile([P, HPC, P], BF16)
            nc.scalar.dma_start(tri_sb[:], tri_in[:])

            # persistent SBUF activations (bf16, feature-major q/k)
            qT = pp.tile([P, ROWS], BF16)
            kT = pp.tile([P, ROWS], BF16)
            # v row-major chunks + ones column for the softmax denominator:
            # vaug[:, ch, h, 0:64] = v rows, vaug[:, ch, h, 64] = 1.0
            vaug = pp.tile([P, NRC * B * 4, HPC, HD + 1], BF16)
            ones_f = pp.tile([P, NRC * B * 4], F32)
            nc.vector.memset(ones_f[:], 1.0)
            nc.vector.tensor_copy(vaug[:, :, 0, HD], ones_f[:])
            nc.vector.tensor_copy(vaug[:, :, 1, HD], ones_f[:])

            # head-selector matrix for the post-a2a 1/s broadcast:
            # e2[i, kk, d] = 1 iff i == 2*kk + (d // 64)
            e2_sb = pp.tile([NH, CK, P], BF16)
            nc.scalar.dma_start(e2_sb[:], e2_in[:])

            # weights ride the scalar DMA queue so the sync queue starts
            # streaming xT immediately
            w_sb = {}
            for nm, wt in (("q", wq_in), ("k", wk_in), ("v", wv_in)):
                wsb = pp.tile([P, CK, D2], BF16, name=f"w_{nm}")
                nc.scalar.dma_start(wsb[:], wt[:])
                w_sb[nm] = wsb
            wp_sb = pp.tile([P, CK, C], BF16)
            nc.scalar.dma_start(wp_sb[:], wp_in[:])

            # ---------------- filler work queue ----------------
            filler = deque()

            def drain(k):
                for _ in range(min(k, len(filler))):
                    filler.popleft()()

            def drain_all():
                while filler:
                    filler.popleft()()

            # ---------------- qkv emission (as filler ops) ----------------
            rc_order = [(b, rc) for rc in range(NRC) for b in range(B)]
            rc_done = [False] * len(rc_order)
            rc_ptr = 0

            def push_qkv_rc(idx):
                """queue qkv + v-transpose for row-chunk rc_order[idx]."""
                b, rc = rc_order[idx]
                n = b * NRC + rc
                n0 = n * QTL
                xts = []

                def op_dma():
                    # one 3D-AP DMA loads all 8 contraction chunks: the
                    # sync queue's ~0.6us per-instruction issue cost is
                    # the scarce resource, not transfer bandwidth
                    xtall = pw.tile([P, CK, QTL], BF16, tag="xt", bufs=3,
                                    name="xt")
                    nc.sync.dma_start(
                        xtall[:],
                        xT_in[:, n0: n0 + QTL].rearrange(
                            "(co p) c -> p co c", p=P),
                    )
                    xts.append(xtall)
                filler.append(op_dma)

                def mk_mm(nm, ko0, holder):
                    def op():
                        if ko0 == 0:
                            holder.append(
                                ps.tile([P, QTL], F32, tag="qkv", bufs=2,
                                        name="acc"))
                        acc = holder[0]
                        for ko in range(ko0, ko0 + 4):
                            nc.tensor.matmul(
                                acc[:], w_sb[nm][:, ko, :], xts[0][:, ko, :],
                                start=(ko == 0), stop=(ko == CK - 1),
                            )
                    return op

                for nm in ("q", "k", "v"):
                    holder = []
                    filler.append(mk_mm(nm, 0, holder))
                    filler.append(mk_mm(nm, 4, holder))
                    if nm != "v":
                        dstT = qT if nm == "q" else kT

                        def op_copy(holder=holder, dstT=dstT):
                            nc.vector.tensor_copy(
                                dstT[:, n0: n0 + QTL], holder[0][:])
                        filler.append(op_copy)
                    else:
                        def op_v(holder=holder):
                            vtmp = pw.tile([P, QTL], BF16, tag="vtmp", bufs=2)
                            nc.vector.tensor_copy(vtmp[:], holder[0][:])
                            tv = ps.tile([P, 4, P], BF16, tag="qkv", bufs=2,
                                         name="tv")
                            for s in range(4):
                                nc.tensor.transpose(
                                    tv[:, s, :], vtmp[:, s * P: (s + 1) * P],
                                    ident[:]
                                )
                            ch0 = 4 * n
                            for h in range(HPC):
                                nc.vector.tensor_copy(
                                    vaug[:, ch0: ch0 + 4, h, 0:HD],
                                    tv[:, :, h * HD: (h + 1) * HD],
                                )
                        filler.append(op_v)

                filler.append(lambda: rc_done.__setitem__(idx, True))

            def ensure_rc(upto):
                nonlocal rc_ptr
                while rc_ptr <= upto:
                    push_qkv_rc(rc_ptr)
                    rc_ptr += 1

            # ---------------- attention tile ----------------
            def emit_attn_tile(b, j, gname, jx):
                """scores+exp+mask+y for q-tile j of batch b, then stage
                the unnormalized y and s rows into group gname's a2a-in."""
                q0 = (b * NJ + j) * QTL
                nkv = 4 * j + 4
                slot0 = 2 * b
                ps_ys = [
                    ps.tile([HD + 1, QTL], F32, tag="y", bufs=2,
                            name=f"ps_y{h}")
                    for h in range(HPC)
                ]
                pend = None  # (i, att, c0)

                def emit_y(item):
                    i, att, c0 = item
                    ch = b * KVC + i
                    for h in range(HPC):
                        nc.tensor.matmul(
                            ps_ys[h][:, c0:QTL],
                            vaug[:, ch, h, :],
                            att[:, h, c0:QTL],
                            start=(i == 0), stop=(i == nkv - 1),
                        )

                for i in range(nkv):
                    ch = b * KVC + i
                    m = i - 4 * j
                    c0 = max(m, 0) * P          # first causal q column
                    ps_s = ps.tile([P, HPC, QTL], F32, tag="s", bufs=2,
                                   name="ps_s")
                    for h in range(HPC):
                        hsl = slice(h * HD, (h + 1) * HD)
                        nc.tensor.matmul(
                            ps_s[:, h, c0:QTL],
                            kT[hsl, ch * P: (ch + 1) * P],
                            qT[hsl, q0 + c0: q0 + QTL],
                            start=True, stop=True,
                        )
                    att = pw.tile([P, HPC, QTL], BF16, tag="att", bufs=3)
                    nc.scalar.activation(
                        att[:, :, c0:QTL], ps_s[:, :, c0:QTL], AF.Exp,
                        scale=float(SCALE),
                    )
                    if m >= 0:
                        nc.vector.tensor_tensor(
                            att[:, :, c0: c0 + P],
                            att[:, :, c0: c0 + P],
                            tri_sb[:],
                            ALU.mult,
                        )
                    if pend is not None:
                        emit_y(pend)
                    pend = (i, att, c0)
                    drain(3)
                emit_y(pend)

                # stage into the a2a input: y rows + s rows, split across
                # 2 col-pieces of QC each
                ybuf = a_in[gname]
                for h in range(HPC):
                    yraw = pw.tile([HD + 1, QTL], BF16, tag=f"yraw{h}",
                                   bufs=2, name=f"yraw{h}")
                    nc.vector.tensor_copy(yraw[:], ps_ys[h][0: HD + 1, :])
                    # y rows: iterate (feat, slot, col) on both sides
                    nc.sync.dma_start(
                        ybuf[slot0: slot0 + 2,
                             h * HD: (h + 1) * HD, jx, :].transpose([1, 0, 2]),
                        yraw[0:HD, :].rearrange("p (s c) -> p s c", s=2),
                    )
                    nc.sync.dma_start(
                        ybuf[slot0: slot0 + 2,
                             D2 + h: D2 + h + 1, jx, :].transpose([1, 0, 2]),
                        yraw[HD: HD + 1, :].rearrange("p (s c) -> p s c", s=2),
                    )

            # ---------------- proj (tail work, FIFO per quarter) --------
            def push_proj(gname, jx, col0):
                aout = a_out[gname]
                srec = []
                yrs = []

                def op_rec():
                    s_sb = pw.tile([NH, QC], BF16, tag="s_sb", bufs=2)
                    for h in range(HPC):
                        nc.sync.dma_start(
                            s_sb[h: NH: HPC, :], aout[:, D2 + h, jx, :])
                    # all 8 cores' y in one [128, 8, QC] tile, one DMA
                    yrall = pw.tile([P, NCORES, QC], BF16, tag="yrall",
                                    bufs=2)
                    nc.sync.dma_start(
                        yrall[:],
                        aout[:, 0:D2, jx, :].transpose([1, 0, 2]))
                    yrs.append(yrall)
                    rec = pw.tile([NH, QC], BF16, tag="rec_sb", bufs=2)
                    with nc.allow_low_precision("1/s at bf16, tol 2e-2"):
                        nc.vector.reciprocal(rec[:], s_sb[:])
                    srec.append(rec)
                filler.append(op_rec)

                yns = []

                def mk_yn(kk):
                    def op():
                        ps_bc = ps.tile([P, QC], F32, tag="s", bufs=2,
                                        name="ps_bc")
                        nc.tensor.matmul(
                            ps_bc[:], e2_sb[:, kk, :], srec[0][:],
                            start=True, stop=True,
                        )
                        yn = pw.tile([P, QC], BF16, tag=f"yn{kk}", bufs=2,
                                     name=f"yn{kk}")
                        nc.vector.tensor_tensor(yn[:], yrs[0][:, kk, :],
                                                ps_bc[:], ALU.mult)
                        yns.append(yn)
                    return op

                for kk in range(NCORES):
                    filler.append(mk_yn(kk))

                def mk_oc(oc):
                    def op():
                        ps_o = ps.tile([P, QC], F32, tag="qkv", bufs=2,
                                       name="ps_o")
                        for kk in range(NCORES):
                            nc.tensor.matmul(
                                ps_o[:],
                                wp_sb[:, kk, oc * P: (oc + 1) * P],
                                yns[kk][:],
                                start=(kk == 0), stop=(kk == NCORES - 1),
                            )
                        osb = pw.tile([P, QC], F32, tag="osb", bufs=2)
                        nc.vector.tensor_copy(osb[:], ps_o[:])
                        # outT rides the scalar queue (idle post-attention)
                        # so it never delays staging DMAs on sync
                        nc.scalar.dma_start(
                            outT[oc * P: (oc + 1) * P, col0: col0 + QC],
                            osb[:],
                        )
                    return op

                for oc in range(CK):
                    filler.append(mk_oc(oc))

            # ---------------- main emission ----------------
            t_idx = 0
            for gname, js, tiles in GROUPS:
                for b, j in tiles:
                    ensure_rc(min(t_idx + 1, len(rc_order) - 1))
                    while not rc_done[t_idx]:
                        drain(1)
                    emit_attn_tile(b, j, gname, j - js[0])
                    t_idx += 1
                nc.gpsimd.collective_compute(
                    "AllToAll", ALU.bypass, replica_groups=rg,
                    ins=[a_in[gname][:].opt()], outs=[a_out[gname][:].opt()],
                )
            # all proj at the tail, in collective-completion order
            for gname, js, _ in GROUPS:
                for jx, j in enumerate(js):
                    push_proj(gname, jx, j * QC)
            drain_all()

    nc.finalize()
    return nc


def _get_nc():
    global _CACHED_NC
    if _CACHED_NC is None:
        _CACHED_NC = _build_nc()
    return _CACHED_NC


def kernel(x, mask, w_qkv, w_proj):
    bf = ml_dtypes.bfloat16
    x = np.asarray(x, dtype=np.float32)
    w_qkv = np.asarray(w_qkv, dtype=np.float32)
    w_proj = np.asarray(w_proj, dtype=np.float32)

    # host-side input layout prep: feature-major bf16 activations and
    # partition-packed weights (w[ko*128+p, d] -> packed[p, ko, d])
    xT = np.ascontiguousarray(x.reshape(ROWS, C).T.astype(bf))

    def pack(w):  # [C, D] -> [P, CK, D], partition-major
        return np.ascontiguousarray(
            w.reshape(CK, P, -1).transpose(1, 0, 2).astype(bf)
        )

    wp_bf = pack(w_proj)

    # diagonal-block causal pattern (multiplicative, transposed):
    # tri[p, h, c] = keep(kv_local=p, q_local=c), identical for both heads
    mt = np.asarray(mask).reshape(T, T)[:P, :P].T.astype(bf)
    tri = np.ascontiguousarray(np.broadcast_to(mt[:, None, :], (P, HPC, P)))

    # head-selector for the post-a2a 1/s broadcast
    e2 = np.zeros((NH, CK, P), dtype=bf)
    for kk in range(CK):
        e2[2 * kk, kk, 0:HD] = 1.0
        e2[2 * kk + 1, kk, HD:P] = 1.0

    in_maps = []
    for r in range(NCORES):
        sl = slice(r * D2, (r + 1) * D2)
        in_maps.append(
            {
                "xT": xT,
                "wq": pack(w_qkv[:, sl]),
                "wk": pack(w_qkv[:, C:][:, sl]),
                "wv": pack(w_qkv[:, 2 * C:][:, sl]),
                "wp": wp_bf,
                "tri": tri,
                "e2": e2,
            }
        )

    nc = _get_nc()
    res = run_bass_kernel_spmd(nc, in_maps, core_ids=list(range(NCORES)))
    global LAST_RESULTS
    LAST_RESULTS = res

    # unshard: core r's outT columns [j*256:(j+1)*256) hold q rows
    # j*512 + (r%2)*256 + [0:256) of batch r//2
    out = np.empty((B, T, C), dtype=np.float32)
    for r in range(NCORES):
        oT = res.results[r]["outT"]  # [C, RSL]
        for j in range(4):
            q0 = j * 512 + (r % 2) * 256
            out[r // 2, q0: q0 + 256, :] = oT[:, j * 256: (j + 1) * 256].T
    return out.reshape(B, T, C)
